# revision 11
# baseline (speedup 1.0000x reference)
"""Llama4-style MoE (top-1 routing, E=8) on 8 Trainium2 NeuronCores.

Sharding (expert-parallel, as in the hint): the router runs on host (it IS the
dispatch that defines the sharding); tokens are gathered per top-1 expert and
shipped, pre-scaled by their sigmoid router score, to the core owning that
expert. The shared expert is token-sharded: core c also runs the shared MLP for
tokens [c*1024, (c+1)*1024). Each core therefore executes two SwiGLU jobs with
identical shape: (expert weights, routed batch) and (shared weights, token
shard). Outputs are gathered and combined host-side (scatter-add), so no
on-device collectives are needed.

Device kernel layout: all activations are [feature(partition), token(free)].
  phase A: G = WgT.T @ X, U = WuT.T @ X (PSUM f32, k-accumulated),
           hm = silu(G) * U  -> SBUF bf16, kept resident for the whole job.
  phase B: Y[h,t] += WdT[m,h].T @ hm[m,t], accumulated over all 32 i-chunks in
           PSUM (8 banks = 1024 h at a time, two h-passes), then copied to SBUF
           and DMA'd out as f32.
Matmuls are bf16 (host-cast) with f32 PSUM accumulation.
"""

import os
import sys
import types

import numpy as np
import ml_dtypes

import concourse.bacc as bacc
import concourse.bass_utils as bass_utils
import concourse.mybir as mybir
import concourse.tile as tile
from concourse.bass_utils import run_bass_kernel_spmd

BF16 = mybir.dt.bfloat16
F32 = mybir.dt.float32

N_CORES = 8
T = 8192
H = 2048
I_DIM = 4096
TSUB = 512  # token sub-chunk = PSUM bank free dim (f32)
USE_SILU = False  # native Silu LUT isn't implemented in CoreSim

_prog_cache: dict = {}


def _ensure_ntff_hook():
    """Register the axon NTFF profiling hook if the image didn't.

    ``bass_utils.run_bass_kernel_spmd(trace=True)`` under axon imports
    ``antenv.axon_hooks``; this container ships only an ``antenv`` stub, but
    the hook implementation exists in ``trn_agent_boot.trn_boot``. Wire the
    two together so we get HW exec times from NTFF. Degrades silently —
    ``run_bass_kernel_spmd`` then runs untraced."""
    try:
        from antenv.axon_hooks import get_axon_ntff_profile_hook  # noqa: F401
        return
    except ImportError:
        pass
    try:
        import antenv
        from trn_agent_boot.trn_boot import _ntff_profile_via_ctypes
        so_path = "/opt/axon/libaxon_pjrt.so"
        if not os.path.exists(so_path):
            return
        hook = _ntff_profile_via_ctypes(so_path)
        mod = types.ModuleType("antenv.axon_hooks")
        mod._hook = hook
        mod.get_axon_ntff_profile_hook = lambda: mod._hook

        def _set(h):
            mod._hook = h

        mod.set_axon_ntff_profile_hook = _set
        sys.modules["antenv.axon_hooks"] = mod
        antenv.axon_hooks = mod
        # The NEFF artifact upload targets a cloud bucket that is not
        # reachable from this sandbox; keep artifacts local instead.
        bass_utils.upload_artifacts = lambda tmpdir: str(tmpdir)
    except Exception:
        pass


def _build(cap: int, tok_shared: int, h: int = H, i_dim: int = I_DIM):
    """Build the per-core SPMD Bass program. All 8 cores run this identically
    on different data. cap/tok_shared are the routed-batch capacity and the
    shared-expert shard size (both multiples of TSUB)."""
    kc = h // 128       # contraction chunks for gate/up
    mc = i_dim // 128   # intermediate chunks
    tot = cap + tok_shared
    oc = h // 128       # 128-row output chunks
    # phase-B passes: up to 8 output chunks (8 PSUM banks) at a time
    passes = [(p0, min(8, oc - p0)) for p0 in range(0, oc, 8)]

    nc = bacc.Bacc("TRN2", target_bir_lowering=False, debug=False,
                   num_devices=N_CORES)

    x_d = nc.dram_tensor("x", [h, tot], BF16, kind="ExternalInput")
    wg_d = nc.dram_tensor("wg", [2, h, i_dim], BF16, kind="ExternalInput")
    wu_d = nc.dram_tensor("wu", [2, h, i_dim], BF16, kind="ExternalInput")
    wd_d = nc.dram_tensor("wd", [2, i_dim, h], BF16, kind="ExternalInput")
    y_d = nc.dram_tensor("y", [h, tot], F32, kind="ExternalOutput")

    x_r = x_d.ap().rearrange("(k p) t -> p k t", p=128)
    wg_r = wg_d.ap().rearrange("j (k p) i -> j p k i", p=128)
    wu_r = wu_d.ap().rearrange("j (k p) i -> j p k i", p=128)
    wd_ap = wd_d.ap()
    y_ap = y_d.ap()

    jobs = [(0, cap), (1, tok_shared)]

    with tile.TileContext(nc) as tc:
        with (
            tc.tile_pool(name="xpool", bufs=1) as pool_x,
            tc.tile_pool(name="hmpool", bufs=1) as pool_hm,
            tc.tile_pool(name="cpool", bufs=1) as pool_c,
        ):
            zb = pool_c.tile([128, 1], F32, name="zb")
            nc.gpsimd.memset(zb[:], 0.0)

            for j, ntok in jobs:
                c0 = 0 if j == 0 else cap
                nt = ntok // TSUB

                x_t = pool_x.tile([128, kc, ntok], BF16, tag="x", name="x_t")
                for k in range(kc):
                    nc.sync.dma_start(out=x_t[:, k, :],
                                      in_=x_r[:, k, c0:c0 + ntok])
                hm = pool_hm.tile([128, mc, ntok], BF16, tag="hm", name="hm")

                # ---- phase A: hm = silu(Wg.T @ x) * (Wu.T @ x) ----
                with (
                    tc.tile_pool(name="psA", bufs=1, space="PSUM") as ps_a,
                    tc.tile_pool(name="wA", bufs=2) as pool_w,
                    tc.tile_pool(name="aA", bufs=3) as pool_a,
                ):
                    for m in range(mc):
                        ms = slice(m * 128, (m + 1) * 128)
                        wg_t = pool_w.tile([128, kc, 128], BF16, tag="wg",
                                           name="wg_t")
                        wu_t = pool_w.tile([128, kc, 128], BF16, tag="wu",
                                           name="wu_t")
                        nc.sync.dma_start(out=wg_t[:], in_=wg_r[j, :, :, ms])
                        nc.sync.dma_start(out=wu_t[:], in_=wu_r[j, :, :, ms])

                        gs = [ps_a.tile([128, TSUB], F32, tag=f"g{t}",
                                        name=f"g{t}") for t in range(nt)]
                        us = [ps_a.tile([128, TSUB], F32, tag=f"u{t}",
                                        name=f"u{t}") for t in range(nt)]
                        for k in range(kc):
                            for t in range(nt):
                                nc.tensor.matmul(
                                    gs[t][:], wg_t[:, k, :],
                                    x_t[:, k, t * TSUB:(t + 1) * TSUB],
                                    start=(k == 0), stop=(k == kc - 1))
                        for k in range(kc):
                            for t in range(nt):
                                nc.tensor.matmul(
                                    us[t][:], wu_t[:, k, :],
                                    x_t[:, k, t * TSUB:(t + 1) * TSUB],
                                    start=(k == 0), stop=(k == kc - 1))
                        for t in range(nt):
                            hm_sl = hm[:, m, t * TSUB:(t + 1) * TSUB]
                            if USE_SILU:
                                a_t = pool_a.tile([128, TSUB], F32, tag="a",
                                                  name="a_t")
                                nc.scalar.activation(
                                    a_t[:], gs[t][:],
                                    mybir.ActivationFunctionType.Silu,
                                    bias=zb[:])
                                nc.vector.tensor_mul(hm_sl, a_t[:], us[t][:])
                            else:
                                # silu(g)*u as sigmoid(g)*g*u (CoreSim has no
                                # Silu; numerically identical up to rounding)
                                a_t = pool_a.tile([128, TSUB], F32, tag="a",
                                                  name="a_t")
                                nc.scalar.activation(
                                    a_t[:], gs[t][:],
                                    mybir.ActivationFunctionType.Sigmoid,
                                    bias=zb[:])
                                w_t = pool_a.tile([128, TSUB], F32, tag="w",
                                                  name="w_t")
                                nc.vector.tensor_mul(w_t[:], a_t[:], gs[t][:])
                                nc.vector.tensor_mul(hm_sl, w_t[:], us[t][:])

                # ---- phase B: y = Wd.T @ hm ----
                with (
                    tc.tile_pool(name="psB", bufs=8, space="PSUM") as ps_b,
                    tc.tile_pool(name="wB", bufs=3) as pool_wd,
                    tc.tile_pool(name="sB", bufs=4) as pool_st,
                ):
                    for t in range(nt):
                        tsl = slice(t * TSUB, (t + 1) * TSUB)
                        for p0, pn in passes:
                            ys = [ps_b.tile([128, TSUB], F32, tag="y",
                                            name=f"y{hh}") for hh in range(pn)]
                            for m in range(mc):
                                wd_t = pool_wd.tile([128, pn * 128], BF16,
                                                    tag="wd", name="wd_t")
                                nc.sync.dma_start(
                                    out=wd_t[:],
                                    in_=wd_ap[j, m * 128:(m + 1) * 128,
                                              p0 * 128:(p0 + pn) * 128])
                                for hh in range(pn):
                                    nc.tensor.matmul(
                                        ys[hh][:],
                                        wd_t[:, hh * 128:(hh + 1) * 128],
                                        hm[:, m, tsl],
                                        start=(m == 0), stop=(m == mc - 1))
                            for hh in range(pn):
                                st = pool_st.tile([128, TSUB], F32, tag="yst",
                                                  name="st")
                                nc.vector.tensor_copy(st[:], ys[hh][:])
                                r0 = (p0 + hh) * 128
                                nc.sync.dma_start(
                                    out=y_ap[r0:r0 + 128, c0 + t * TSUB:
                                             c0 + (t + 1) * TSUB],
                                    in_=st[:])

    nc.compile()
    return nc


def _get_prog(cap: int, tok_shared: int):
    key = (cap, tok_shared)
    if key not in _prog_cache:
        _prog_cache[key] = _build(cap, tok_shared)
    return _prog_cache[key]


def kernel(hidden_states, router_w, w_gate, w_up, w_down,
           sw_gate, sw_up, sw_down):
    hs = np.asarray(hidden_states, np.float32)
    rw = np.asarray(router_w, np.float32)
    w_gate = np.asarray(w_gate, np.float32)
    w_up = np.asarray(w_up, np.float32)
    w_down = np.asarray(w_down, np.float32)
    sw_gate = np.asarray(sw_gate, np.float32)
    sw_up = np.asarray(sw_up, np.float32)
    sw_down = np.asarray(sw_down, np.float32)

    t_tok, h = hs.shape
    e, i_dim = w_gate.shape[0], w_gate.shape[1]
    assert (t_tok, h, i_dim, e) == (T, H, I_DIM, N_CORES)

    # Router on host — this computes the dispatch that defines the sharding.
    # top-1 argmax matches jax.lax.top_k(k=1) (ties -> lowest index).
    logits = hs @ rw.T
    top = np.argmax(logits, axis=1)
    sel = logits[np.arange(t_tok), top].astype(np.float64)
    score = (1.0 / (1.0 + np.exp(-sel))).astype(np.float32)

    order = np.argsort(top, kind="stable")
    counts = np.bincount(top, minlength=e)
    offs = np.zeros(e + 1, np.int64)
    np.cumsum(counts, out=offs[1:])
    cap = max(TSUB, int(-(-int(counts.max()) // TSUB)) * TSUB)
    tok_shared = t_tok // N_CORES
    tot = cap + tok_shared

    nc = _get_prog(cap, tok_shared)

    bf = ml_dtypes.bfloat16
    in_maps = []
    tok_idx = []
    for c in range(N_CORES):
        idx = order[offs[c]:offs[c + 1]]
        tok_idx.append(idx)
        x = np.zeros((h, tot), bf)
        x[:, :len(idx)] = (hs[idx] * score[idx, None]).T.astype(bf)
        x[:, cap:] = hs[c * tok_shared:(c + 1) * tok_shared].T.astype(bf)
        wg = np.empty((2, h, i_dim), bf)
        wg[0] = w_gate[c].T
        wg[1] = sw_gate.T
        wu = np.empty((2, h, i_dim), bf)
        wu[0] = w_up[c].T
        wu[1] = sw_up.T
        wd = np.empty((2, i_dim, h), bf)
        wd[0] = w_down[c].T
        wd[1] = sw_down.T
        in_maps.append({"x": x, "wg": wg, "wu": wu, "wd": wd})

    _ensure_ntff_hook()
    tmpdir = os.environ.get("KERNEL_TRACE_DIR") or None
    if tmpdir:
        os.makedirs(tmpdir, exist_ok=True)
    try:
        res = run_bass_kernel_spmd(nc, in_maps, list(range(N_CORES)),
                                   trace=True, tmpdir=tmpdir)
    except Exception as exc:  # profiling plumbing must never break results
        sys.stderr.write(f"traced run failed ({exc!r}); retrying untraced\n")
        res = run_bass_kernel_spmd(nc, in_maps, list(range(N_CORES)),
                                   trace=False)
    kernel.last_exec_time_ns = res.exec_time_ns
    kernel.last_results = res
    kernel.last_in_maps = in_maps
    kernel.last_meta = (cap, tok_shared, tok_idx)

    out = np.empty((t_tok, h), np.float32)
    ys = [np.asarray(res.results[c]["y"], np.float32) for c in range(N_CORES)]
    for c in range(N_CORES):  # shared-expert shards first...
        out[c * tok_shared:(c + 1) * tok_shared] = ys[c][:, cap:].T
    for c in range(N_CORES):  # ...then scatter-add every routed batch
        idx = tok_idx[c]
        if len(idx):
            out[idx] += ys[c][:, :len(idx)].T
    return out


kernel.last_exec_time_ns = None
kernel.last_results = None


# revision 14
# speedup vs baseline: 1.1509x; 1.1509x over previous
"""Llama4-style MoE (top-1 routing, E=8) on 8 Trainium2 NeuronCores.

Sharding (expert-parallel, as in the hint): the router runs on host (it IS the
dispatch that defines the sharding); tokens are gathered per top-1 expert and
shipped, pre-scaled by their sigmoid router score, to the core owning that
expert. The shared expert is token-sharded: core c also runs the shared MLP for
tokens [c*1024, (c+1)*1024). Each core therefore executes two SwiGLU jobs with
identical shape: (expert weights, routed batch) and (shared weights, token
shard). Outputs are gathered and combined host-side (scatter-add), so no
on-device collectives are needed.

Device kernel layout: all activations are [feature(partition), token(free)].
  phase A: G = WgT.T @ X, U = WuT.T @ X (PSUM f32, k-accumulated),
           hm = silu(G) * U  -> SBUF bf16, kept resident for the whole job.
  phase B: Y[h,t] += WdT[m,h].T @ hm[m,t], accumulated over all 32 i-chunks in
           PSUM (8 banks = 1024 h at a time, two h-passes), then copied to SBUF
           and DMA'd out as f32.
Matmuls are bf16 (host-cast) with f32 PSUM accumulation.
"""

import os
import sys
import types

import numpy as np
import ml_dtypes

import concourse.bacc as bacc
import concourse.bass_utils as bass_utils
import concourse.mybir as mybir
import concourse.tile as tile
from concourse.bass_utils import run_bass_kernel_spmd

BF16 = mybir.dt.bfloat16
F32 = mybir.dt.float32

N_CORES = 8
T = 8192
H = 2048
I_DIM = 4096
TSUB = 512  # token sub-chunk = PSUM bank free dim (f32)
USE_SILU = False  # native Silu LUT isn't implemented in CoreSim

_prog_cache: dict = {}


def _ensure_ntff_hook():
    """Register the axon NTFF profiling hook if the image didn't.

    ``bass_utils.run_bass_kernel_spmd(trace=True)`` under axon imports
    ``antenv.axon_hooks``; this container ships only an ``antenv`` stub, but
    the hook implementation exists in ``trn_agent_boot.trn_boot``. Wire the
    two together so we get HW exec times from NTFF. Degrades silently —
    ``run_bass_kernel_spmd`` then runs untraced."""
    try:
        from antenv.axon_hooks import get_axon_ntff_profile_hook  # noqa: F401
        return
    except ImportError:
        pass
    try:
        import antenv
        from trn_agent_boot.trn_boot import _ntff_profile_via_ctypes
        so_path = "/opt/axon/libaxon_pjrt.so"
        if not os.path.exists(so_path):
            return
        hook = _ntff_profile_via_ctypes(so_path)
        mod = types.ModuleType("antenv.axon_hooks")
        mod._hook = hook
        mod.get_axon_ntff_profile_hook = lambda: mod._hook

        def _set(h):
            mod._hook = h

        mod.set_axon_ntff_profile_hook = _set
        sys.modules["antenv.axon_hooks"] = mod
        antenv.axon_hooks = mod
        # The NEFF artifact upload targets a cloud bucket that is not
        # reachable from this sandbox; keep artifacts local instead.
        bass_utils.upload_artifacts = lambda tmpdir: str(tmpdir)
    except Exception:
        pass


def _chunks(ntok: int):
    """Token sub-chunks: full 512s plus one tail (multiple of 64)."""
    out = []
    o = 0
    while o + TSUB <= ntok:
        out.append((o, TSUB))
        o += TSUB
    if o < ntok:
        out.append((o, ntok - o))
    return out


def _build(cap: int, tok_shared: int, h: int = H, i_dim: int = I_DIM):
    """Build the per-core SPMD Bass program. All 8 cores run this identically
    on different data. cap/tok_shared are the routed-batch capacity (multiple
    of 64) and the shared-expert shard size."""
    kc = h // 128       # contraction chunks for gate/up
    mc = i_dim // 128   # intermediate chunks
    tot = cap + tok_shared
    oc = h // 128       # 128-row output chunks
    # phase-B passes: up to 8 output chunks (8 PSUM banks) at a time
    passes = [(p0, min(8, oc - p0)) for p0 in range(0, oc, 8)]

    nc = bacc.Bacc("TRN2", target_bir_lowering=False, debug=False,
                   num_devices=N_CORES)

    x_d = nc.dram_tensor("x", [h, tot], BF16, kind="ExternalInput")
    wg_d = nc.dram_tensor("wg", [2, h, i_dim], BF16, kind="ExternalInput")
    wu_d = nc.dram_tensor("wu", [2, h, i_dim], BF16, kind="ExternalInput")
    wd_d = nc.dram_tensor("wd", [2, i_dim, h], BF16, kind="ExternalInput")
    y_d = nc.dram_tensor("y", [h, tot], F32, kind="ExternalOutput")

    x_r = x_d.ap().rearrange("(k p) t -> p k t", p=128)
    wg_r = wg_d.ap().rearrange("j (k p) i -> j p k i", p=128)
    wu_r = wu_d.ap().rearrange("j (k p) i -> j p k i", p=128)
    wd_ap = wd_d.ap()
    y_ap = y_d.ap()

    jobs = [(0, 0, cap), (1, cap, tok_shared)]  # (j, col0, ntok)

    with tile.TileContext(nc) as tc:
        with (
            tc.tile_pool(name="xpool", bufs=1) as pool_x,
            tc.tile_pool(name="hmpool", bufs=1) as pool_hm,
            tc.tile_pool(name="cpool", bufs=1) as pool_c,
            tc.tile_pool(name="wA", bufs=3) as pool_w,
            tc.tile_pool(name="aA", bufs=2) as pool_a,
            tc.tile_pool(name="wB", bufs=3) as pool_wd,
            tc.tile_pool(name="sB", bufs=4) as pool_st,
        ):
            zb = pool_c.tile([128, 1], F32, name="zb")
            nc.gpsimd.memset(zb[:], 0.0)

            xts = {}
            w0 = {}

            def prefetch(j, c0, ntok):
                # First i-chunk's weights before x so PE can start ~immediately
                wg_t = pool_w.tile([128, kc, 128], BF16, tag="wg", name="wg_t")
                wu_t = pool_w.tile([128, kc, 128], BF16, tag="wu", name="wu_t")
                nc.sync.dma_start(out=wg_t[:], in_=wg_r[j, :, :, 0:128])
                nc.sync.dma_start(out=wu_t[:], in_=wu_r[j, :, :, 0:128])
                w0[j] = (wg_t, wu_t)
                x_t = pool_x.tile([128, kc, ntok], BF16, tag="x", name="x_t")
                for k in range(kc):
                    nc.sync.dma_start(out=x_t[:, k, :],
                                      in_=x_r[:, k, c0:c0 + ntok])
                xts[j] = x_t

            def phase_a(j, ntok, x_t, hm):
                cks = _chunks(ntok)
                with tc.tile_pool(name="psA", bufs=1, space="PSUM") as ps_a:
                    for m in range(mc):
                        if m == 0:
                            wg_t, wu_t = w0[j]
                        else:
                            ms = slice(m * 128, (m + 1) * 128)
                            wg_t = pool_w.tile([128, kc, 128], BF16, tag="wg",
                                               name="wg_t")
                            wu_t = pool_w.tile([128, kc, 128], BF16, tag="wu",
                                               name="wu_t")
                            nc.sync.dma_start(out=wg_t[:],
                                              in_=wg_r[j, :, :, ms])
                            nc.sync.dma_start(out=wu_t[:],
                                              in_=wu_r[j, :, :, ms])
                        gs = [ps_a.tile([128, sz], F32, tag=f"g{t}",
                                        name=f"g{t}")
                              for t, (_, sz) in enumerate(cks)]
                        us = [ps_a.tile([128, sz], F32, tag=f"u{t}",
                                        name=f"u{t}")
                              for t, (_, sz) in enumerate(cks)]
                        for k in range(kc):
                            for t, (o, sz) in enumerate(cks):
                                nc.tensor.matmul(
                                    gs[t][:], wg_t[:, k, :],
                                    x_t[:, k, o:o + sz],
                                    start=(k == 0), stop=(k == kc - 1))
                        for k in range(kc):
                            for t, (o, sz) in enumerate(cks):
                                nc.tensor.matmul(
                                    us[t][:], wu_t[:, k, :],
                                    x_t[:, k, o:o + sz],
                                    start=(k == 0), stop=(k == kc - 1))
                        for t, (o, sz) in enumerate(cks):
                            hm_sl = hm[:, m, o:o + sz]
                            a_t = pool_a.tile([128, sz], F32, tag=f"a{t}",
                                              name="a_t")
                            if USE_SILU:
                                nc.scalar.activation(
                                    a_t[:], gs[t][:],
                                    mybir.ActivationFunctionType.Silu,
                                    bias=zb[:])
                                nc.vector.tensor_mul(hm_sl, a_t[:], us[t][:])
                            else:
                                # silu(g)*u as sigmoid(g)*g*u (CoreSim has no
                                # Silu LUT; identical up to rounding)
                                nc.scalar.activation(
                                    a_t[:], gs[t][:],
                                    mybir.ActivationFunctionType.Sigmoid,
                                    bias=zb[:])
                                w_t = pool_a.tile([128, sz], F32, tag=f"w{t}",
                                                  name="w_t")
                                nc.vector.tensor_mul(w_t[:], a_t[:], gs[t][:])
                                nc.vector.tensor_mul(hm_sl, w_t[:], us[t][:])

            def phase_b(j, c0, ntok, hm):
                cks = _chunks(ntok)
                with tc.tile_pool(name="psB", bufs=8, space="PSUM") as ps_b:
                    for t, (o, sz) in enumerate(cks):
                        tsl = slice(o, o + sz)
                        for p0, pn in passes:
                            ys = [ps_b.tile([128, sz], F32, tag="y",
                                            name=f"y{hh}") for hh in range(pn)]
                            for m in range(mc):
                                wd_t = pool_wd.tile([128, pn * 128], BF16,
                                                    tag="wd", name="wd_t")
                                nc.sync.dma_start(
                                    out=wd_t[:],
                                    in_=wd_ap[j, m * 128:(m + 1) * 128,
                                              p0 * 128:(p0 + pn) * 128])
                                for hh in range(pn):
                                    nc.tensor.matmul(
                                        ys[hh][:],
                                        wd_t[:, hh * 128:(hh + 1) * 128],
                                        hm[:, m, tsl],
                                        start=(m == 0), stop=(m == mc - 1))
                            for hh in range(pn):
                                st = pool_st.tile([128, sz], F32, tag="yst",
                                                  name="st")
                                nc.vector.tensor_copy(st[:], ys[hh][:])
                                r0 = (p0 + hh) * 128
                                nc.sync.dma_start(
                                    out=y_ap[r0:r0 + 128, c0 + o:c0 + o + sz],
                                    in_=st[:])

            prefetch(*jobs[0])
            for ji, (j, c0, ntok) in enumerate(jobs):
                hm = pool_hm.tile([128, mc, ntok], BF16, tag="hm", name="hm")
                phase_a(j, ntok, xts[j], hm)
                if ji + 1 < len(jobs):
                    prefetch(*jobs[ji + 1])
                phase_b(j, c0, ntok, hm)

    nc.compile()
    return nc


def _get_prog(cap: int, tok_shared: int):
    key = (cap, tok_shared)
    if key not in _prog_cache:
        _prog_cache[key] = _build(cap, tok_shared)
    return _prog_cache[key]


def kernel(hidden_states, router_w, w_gate, w_up, w_down,
           sw_gate, sw_up, sw_down):
    hs = np.asarray(hidden_states, np.float32)
    rw = np.asarray(router_w, np.float32)
    w_gate = np.asarray(w_gate, np.float32)
    w_up = np.asarray(w_up, np.float32)
    w_down = np.asarray(w_down, np.float32)
    sw_gate = np.asarray(sw_gate, np.float32)
    sw_up = np.asarray(sw_up, np.float32)
    sw_down = np.asarray(sw_down, np.float32)

    t_tok, h = hs.shape
    e, i_dim = w_gate.shape[0], w_gate.shape[1]
    assert (t_tok, h, i_dim, e) == (T, H, I_DIM, N_CORES)

    # Router on host — this computes the dispatch that defines the sharding.
    # top-1 argmax matches jax.lax.top_k(k=1) (ties -> lowest index).
    logits = hs @ rw.T
    top = np.argmax(logits, axis=1)
    sel = logits[np.arange(t_tok), top].astype(np.float64)
    score = (1.0 / (1.0 + np.exp(-sel))).astype(np.float32)

    order = np.argsort(top, kind="stable")
    counts = np.bincount(top, minlength=e)
    offs = np.zeros(e + 1, np.int64)
    np.cumsum(counts, out=offs[1:])
    cap = max(64, -(-int(counts.max()) // 64) * 64)
    tok_shared = t_tok // N_CORES
    tot = cap + tok_shared

    nc = _get_prog(cap, tok_shared)

    bf = ml_dtypes.bfloat16
    in_maps = []
    tok_idx = []
    for c in range(N_CORES):
        idx = order[offs[c]:offs[c + 1]]
        tok_idx.append(idx)
        x = np.zeros((h, tot), bf)
        x[:, :len(idx)] = (hs[idx] * score[idx, None]).T.astype(bf)
        x[:, cap:] = hs[c * tok_shared:(c + 1) * tok_shared].T.astype(bf)
        wg = np.empty((2, h, i_dim), bf)
        wg[0] = w_gate[c].T
        wg[1] = sw_gate.T
        wu = np.empty((2, h, i_dim), bf)
        wu[0] = w_up[c].T
        wu[1] = sw_up.T
        wd = np.empty((2, i_dim, h), bf)
        wd[0] = w_down[c].T
        wd[1] = sw_down.T
        in_maps.append({"x": x, "wg": wg, "wu": wu, "wd": wd})

    _ensure_ntff_hook()
    tmpdir = os.environ.get("KERNEL_TRACE_DIR") or None
    if tmpdir:
        os.makedirs(tmpdir, exist_ok=True)
    try:
        res = run_bass_kernel_spmd(nc, in_maps, list(range(N_CORES)),
                                   trace=True, tmpdir=tmpdir)
    except Exception as exc:  # profiling plumbing must never break results
        sys.stderr.write(f"traced run failed ({exc!r}); retrying untraced\n")
        res = run_bass_kernel_spmd(nc, in_maps, list(range(N_CORES)),
                                   trace=False)
    kernel.last_exec_time_ns = res.exec_time_ns
    kernel.last_results = res
    kernel.last_in_maps = in_maps
    kernel.last_meta = (cap, tok_shared, tok_idx)

    out = np.empty((t_tok, h), np.float32)
    ys = [np.asarray(res.results[c]["y"], np.float32) for c in range(N_CORES)]
    for c in range(N_CORES):  # shared-expert shards first...
        out[c * tok_shared:(c + 1) * tok_shared] = ys[c][:, cap:].T
    for c in range(N_CORES):  # ...then scatter-add every routed batch
        idx = tok_idx[c]
        if len(idx):
            out[idx] += ys[c][:, :len(idx)].T
    return out


kernel.last_exec_time_ns = None
kernel.last_results = None


# revision 19
# speedup vs baseline: 1.1675x; 1.0145x over previous
"""Llama4-style MoE (top-1 routing, E=8) on 8 Trainium2 NeuronCores.

Sharding (expert-parallel, as in the hint): the router runs on host (it IS the
dispatch that defines the sharding); tokens are gathered per top-1 expert and
shipped, pre-scaled by their sigmoid router score, to the core owning that
expert. The shared expert is token-sharded: core c also runs the shared MLP for
tokens [c*1024, (c+1)*1024). Each core therefore executes two SwiGLU jobs with
identical shape: (expert weights, routed batch) and (shared weights, token
shard). Outputs are gathered and combined host-side (scatter-add), so no
on-device collectives are needed.

Device kernel layout: all activations are [feature(partition), token(free)].
  phase A: G = WgT.T @ X, U = WuT.T @ X (PSUM f32, k-accumulated),
           hm = silu(G) * U  -> SBUF bf16, kept resident for the whole job.
  phase B: Y[h,t] += WdT[m,h].T @ hm[m,t], accumulated over all 32 i-chunks in
           PSUM (8 banks = 1024 h at a time, two h-passes), then copied to SBUF
           and DMA'd out as f32.
Matmuls are bf16 (host-cast) with f32 PSUM accumulation.
"""

import os
import sys
import types

import numpy as np
import ml_dtypes

import concourse.bacc as bacc
import concourse.bass_utils as bass_utils
import concourse.mybir as mybir
import concourse.tile as tile
from concourse.bass_utils import run_bass_kernel_spmd

BF16 = mybir.dt.bfloat16
F32 = mybir.dt.float32

N_CORES = 8
T = 8192
H = 2048
I_DIM = 4096
TSUB = 512  # token sub-chunk = PSUM bank free dim (f32)
USE_SILU = False  # native Silu LUT isn't implemented in CoreSim

_prog_cache: dict = {}


def _ensure_ntff_hook():
    """Register the axon NTFF profiling hook if the image didn't.

    ``bass_utils.run_bass_kernel_spmd(trace=True)`` under axon imports
    ``antenv.axon_hooks``; this container ships only an ``antenv`` stub, but
    the hook implementation exists in ``trn_agent_boot.trn_boot``. Wire the
    two together so we get HW exec times from NTFF. Degrades silently —
    ``run_bass_kernel_spmd`` then runs untraced."""
    try:
        from antenv.axon_hooks import get_axon_ntff_profile_hook  # noqa: F401
        return
    except ImportError:
        pass
    try:
        import antenv
        from trn_agent_boot.trn_boot import _ntff_profile_via_ctypes
        so_path = "/opt/axon/libaxon_pjrt.so"
        if not os.path.exists(so_path):
            return
        hook = _ntff_profile_via_ctypes(so_path)
        mod = types.ModuleType("antenv.axon_hooks")
        mod._hook = hook
        mod.get_axon_ntff_profile_hook = lambda: mod._hook

        def _set(h):
            mod._hook = h

        mod.set_axon_ntff_profile_hook = _set
        sys.modules["antenv.axon_hooks"] = mod
        antenv.axon_hooks = mod
        # The NEFF artifact upload targets a cloud bucket that is not
        # reachable from this sandbox; keep artifacts local instead.
        bass_utils.upload_artifacts = lambda tmpdir: str(tmpdir)
    except Exception:
        pass


def _chunks(ntok: int):
    """Token sub-chunks: full 512s plus one tail (multiple of 64)."""
    out = []
    o = 0
    while o + TSUB <= ntok:
        out.append((o, TSUB))
        o += TSUB
    if o < ntok:
        out.append((o, ntok - o))
    return out


def _build(cap: int, tok_shared: int, h: int = H, i_dim: int = I_DIM):
    """Build the per-core SPMD Bass program. All 8 cores run this identically
    on different data. cap/tok_shared are the routed-batch capacity (multiple
    of 64) and the shared-expert shard size."""
    kc = h // 128       # contraction chunks for gate/up
    mc = i_dim // 128   # intermediate chunks
    tot = cap + tok_shared
    oc = h // 128       # 128-row output chunks
    # phase-B passes: up to 8 output chunks (8 PSUM banks) at a time
    passes = [(p0, min(8, oc - p0)) for p0 in range(0, oc, 8)]

    nc = bacc.Bacc("TRN2", target_bir_lowering=False, debug=False,
                   num_devices=N_CORES)

    x_d = nc.dram_tensor("x", [h, tot], BF16, kind="ExternalInput")
    wg_d = nc.dram_tensor("wg", [2, h, i_dim], BF16, kind="ExternalInput")
    wu_d = nc.dram_tensor("wu", [2, h, i_dim], BF16, kind="ExternalInput")
    wd_d = nc.dram_tensor("wd", [2, i_dim, h], BF16, kind="ExternalInput")
    y_d = nc.dram_tensor("y", [h, tot], F32, kind="ExternalOutput")

    x_r = x_d.ap().rearrange("(k p) t -> p k t", p=128)
    wg_r = wg_d.ap().rearrange("j (k p) i -> j p k i", p=128)
    wu_r = wu_d.ap().rearrange("j (k p) i -> j p k i", p=128)
    wd_ap = wd_d.ap()
    y_ap = y_d.ap()
    y_r = y_d.ap().rearrange("(c p) t -> p c t", p=128)

    jobs = [(0, 0, cap), (1, cap, tok_shared)]  # (j, col0, ntok)

    with tile.TileContext(nc) as tc:
        with (
            tc.tile_pool(name="xpool", bufs=1) as pool_x,
            tc.tile_pool(name="hmpool", bufs=1) as pool_hm,
            tc.tile_pool(name="cpool", bufs=1) as pool_c,
            tc.tile_pool(name="wA", bufs=3) as pool_w,
            tc.tile_pool(name="aA", bufs=2) as pool_a,
            tc.tile_pool(name="wB", bufs=3) as pool_wd,
            tc.tile_pool(name="sB", bufs=4) as pool_st,
        ):
            zb = pool_c.tile([128, 1], F32, name="zb")
            nc.gpsimd.memset(zb[:], 0.0)

            xts = {}
            w0 = {}

            def prefetch(j, c0, ntok):
                # First i-chunk's weights before x so PE can start ~immediately
                wg_t = pool_w.tile([128, kc, 128], BF16, tag="wg", name="wg_t")
                wu_t = pool_w.tile([128, kc, 128], BF16, tag="wu", name="wu_t")
                nc.sync.dma_start(out=wg_t[:], in_=wg_r[j, :, :, 0:128])
                nc.sync.dma_start(out=wu_t[:], in_=wu_r[j, :, :, 0:128])
                w0[j] = (wg_t, wu_t)
                x_t = pool_x.tile([128, kc, ntok], BF16, tag="x", name="x_t")
                for k in range(kc):
                    nc.sync.dma_start(out=x_t[:, k, :],
                                      in_=x_r[:, k, c0:c0 + ntok])
                xts[j] = x_t

            def phase_a(j, ntok, x_t, hm):
                cks = _chunks(ntok)
                with tc.tile_pool(name="psA", bufs=1, space="PSUM") as ps_a:
                    for m in range(mc):
                        if m == 0:
                            wg_t, wu_t = w0[j]
                        else:
                            ms = slice(m * 128, (m + 1) * 128)
                            wg_t = pool_w.tile([128, kc, 128], BF16, tag="wg",
                                               name="wg_t")
                            wu_t = pool_w.tile([128, kc, 128], BF16, tag="wu",
                                               name="wu_t")
                            nc.sync.dma_start(out=wg_t[:],
                                              in_=wg_r[j, :, :, ms])
                            nc.sync.dma_start(out=wu_t[:],
                                              in_=wu_r[j, :, :, ms])
                        gs = [ps_a.tile([128, sz], F32, tag=f"g{t}",
                                        name=f"g{t}")
                              for t, (_, sz) in enumerate(cks)]
                        us = [ps_a.tile([128, sz], F32, tag=f"u{t}",
                                        name=f"u{t}")
                              for t, (_, sz) in enumerate(cks)]
                        for k in range(kc):
                            for t, (o, sz) in enumerate(cks):
                                nc.tensor.matmul(
                                    gs[t][:], wg_t[:, k, :],
                                    x_t[:, k, o:o + sz],
                                    start=(k == 0), stop=(k == kc - 1))
                        for k in range(kc):
                            for t, (o, sz) in enumerate(cks):
                                nc.tensor.matmul(
                                    us[t][:], wu_t[:, k, :],
                                    x_t[:, k, o:o + sz],
                                    start=(k == 0), stop=(k == kc - 1))
                        for t, (o, sz) in enumerate(cks):
                            hm_sl = hm[:, m, o:o + sz]
                            a_t = pool_a.tile([128, sz], F32, tag=f"a{t}",
                                              name="a_t")
                            if USE_SILU:
                                nc.scalar.activation(
                                    a_t[:], gs[t][:],
                                    mybir.ActivationFunctionType.Silu,
                                    bias=zb[:])
                                nc.vector.tensor_mul(hm_sl, a_t[:], us[t][:])
                            else:
                                # silu(g)*u as sigmoid(g)*g*u (CoreSim has no
                                # Silu LUT; identical up to rounding)
                                nc.scalar.activation(
                                    a_t[:], gs[t][:],
                                    mybir.ActivationFunctionType.Sigmoid,
                                    bias=zb[:])
                                w_t = pool_a.tile([128, sz], F32, tag=f"w{t}",
                                                  name="w_t")
                                nc.vector.tensor_mul(w_t[:], a_t[:], gs[t][:])
                                nc.vector.tensor_mul(hm_sl, w_t[:], us[t][:])

            def phase_b(j, c0, ntok, hm):
                # One wd streaming sweep covers ALL token-chunks of the job:
                # per h-pass of pw 128-row output chunks, every token chunk
                # accumulates over m. PSUM: one bank per (chunk, h-slot).
                cks = _chunks(ntok)
                with tc.tile_pool(name="psB", bufs=1, space="PSUM") as ps_b:
                    _phase_b_body(j, c0, cks, hm, ps_b)

            def _phase_b_body(j, c0, cks, hm, ps_b):
                for g0 in range(0, len(cks), 8):
                    grp = cks[g0:g0 + 8]
                    pw = 8 // len(grp)
                    for p0 in range(0, oc, pw):
                        pn = min(pw, oc - p0)
                        ys = [[ps_b.tile([128, sz], F32, tag=f"yf{ci}_{hh}",
                                         name=f"yf{ci}_{hh}")
                               for hh in range(pn)]
                              for ci, (o, sz) in enumerate(grp)]
                        for m in range(mc):
                            wd_t = pool_wd.tile([128, pn * 128], BF16,
                                                tag="wd", name="wd_t")
                            nc.sync.dma_start(
                                out=wd_t[:],
                                in_=wd_ap[j, m * 128:(m + 1) * 128,
                                          p0 * 128:(p0 + pn) * 128])
                            for hh in range(pn):
                                for ci, (o, sz) in enumerate(grp):
                                    nc.tensor.matmul(
                                        ys[ci][hh][:],
                                        wd_t[:, hh * 128:(hh + 1) * 128],
                                        hm[:, m, o:o + sz],
                                        start=(m == 0), stop=(m == mc - 1))
                        for ci, (o, sz) in enumerate(grp):
                            for hh in range(pn):
                                st = pool_st.tile([128, sz], F32,
                                                  tag="yst", name="st")
                                nc.vector.tensor_copy(st[:], ys[ci][hh][:])
                                nc.sync.dma_start(
                                    out=y_r[:, p0 + hh, c0 + o:c0 + o + sz],
                                    in_=st[:])

            prefetch(*jobs[0])
            for ji, (j, c0, ntok) in enumerate(jobs):
                hm = pool_hm.tile([128, mc, ntok], BF16, tag="hm", name="hm")
                phase_a(j, ntok, xts[j], hm)
                if ji + 1 < len(jobs):
                    prefetch(*jobs[ji + 1])
                phase_b(j, c0, ntok, hm)

    nc.compile()
    return nc


def _get_prog(cap: int, tok_shared: int):
    key = (cap, tok_shared)
    if key not in _prog_cache:
        _prog_cache[key] = _build(cap, tok_shared)
    return _prog_cache[key]


def kernel(hidden_states, router_w, w_gate, w_up, w_down,
           sw_gate, sw_up, sw_down):
    hs = np.asarray(hidden_states, np.float32)
    rw = np.asarray(router_w, np.float32)
    w_gate = np.asarray(w_gate, np.float32)
    w_up = np.asarray(w_up, np.float32)
    w_down = np.asarray(w_down, np.float32)
    sw_gate = np.asarray(sw_gate, np.float32)
    sw_up = np.asarray(sw_up, np.float32)
    sw_down = np.asarray(sw_down, np.float32)

    t_tok, h = hs.shape
    e, i_dim = w_gate.shape[0], w_gate.shape[1]
    assert (t_tok, h, i_dim, e) == (T, H, I_DIM, N_CORES)

    # Router on host — this computes the dispatch that defines the sharding.
    # top-1 argmax matches jax.lax.top_k(k=1) (ties -> lowest index).
    logits = hs @ rw.T
    top = np.argmax(logits, axis=1)
    sel = logits[np.arange(t_tok), top].astype(np.float64)
    score = (1.0 / (1.0 + np.exp(-sel))).astype(np.float32)

    order = np.argsort(top, kind="stable")
    counts = np.bincount(top, minlength=e)
    offs = np.zeros(e + 1, np.int64)
    np.cumsum(counts, out=offs[1:])
    cap = max(64, -(-int(counts.max()) // 64) * 64)
    tok_shared = t_tok // N_CORES
    tot = cap + tok_shared

    nc = _get_prog(cap, tok_shared)

    bf = ml_dtypes.bfloat16
    in_maps = []
    tok_idx = []
    for c in range(N_CORES):
        idx = order[offs[c]:offs[c + 1]]
        tok_idx.append(idx)
        x = np.zeros((h, tot), bf)
        x[:, :len(idx)] = (hs[idx] * score[idx, None]).T.astype(bf)
        x[:, cap:] = hs[c * tok_shared:(c + 1) * tok_shared].T.astype(bf)
        wg = np.empty((2, h, i_dim), bf)
        wg[0] = w_gate[c].T
        wg[1] = sw_gate.T
        wu = np.empty((2, h, i_dim), bf)
        wu[0] = w_up[c].T
        wu[1] = sw_up.T
        wd = np.empty((2, i_dim, h), bf)
        wd[0] = w_down[c].T
        wd[1] = sw_down.T
        in_maps.append({"x": x, "wg": wg, "wu": wu, "wd": wd})

    _ensure_ntff_hook()
    tmpdir = os.environ.get("KERNEL_TRACE_DIR") or None
    if tmpdir:
        os.makedirs(tmpdir, exist_ok=True)
    try:
        res = run_bass_kernel_spmd(nc, in_maps, list(range(N_CORES)),
                                   trace=True, tmpdir=tmpdir)
    except Exception as exc:  # profiling plumbing must never break results
        sys.stderr.write(f"traced run failed ({exc!r}); retrying untraced\n")
        res = run_bass_kernel_spmd(nc, in_maps, list(range(N_CORES)),
                                   trace=False)
    kernel.last_exec_time_ns = res.exec_time_ns
    kernel.last_results = res
    kernel.last_in_maps = in_maps
    kernel.last_meta = (cap, tok_shared, tok_idx)

    out = np.empty((t_tok, h), np.float32)
    ys = [np.asarray(res.results[c]["y"], np.float32) for c in range(N_CORES)]
    for c in range(N_CORES):  # shared-expert shards first...
        out[c * tok_shared:(c + 1) * tok_shared] = ys[c][:, cap:].T
    for c in range(N_CORES):  # ...then scatter-add every routed batch
        idx = tok_idx[c]
        if len(idx):
            out[idx] += ys[c][:, :len(idx)].T
    return out


kernel.last_exec_time_ns = None
kernel.last_results = None
